# revision 30
# baseline (speedup 1.0000x reference)
"""MoE router kernel for Trainium2 (Bass/Tile), SPMD over 8 NeuronCores.

Computes, for x:(B,T,D) f32, W:(E,D) f32, x_mask:(B,T) i32 {0,1}:
  m       = x_mask[..., None]
  logits  = (x*m) @ W.T * m            # (B,T,E)
  probs   = softmax(logits, -1)
  ew, ei  = top2(probs);  ew /= ew.sum(-1, keepdims=True);  ew *= m
  ei      = where(m, ei, -1)
  probs   = probs * m
returns (ew, ei, logits, probs).

Sharding: data-parallel over B*T tokens, 4096 tokens per core, W replicated.

Strategy:
- Host pre-transposes each core's x shard to d-major and splits it into an
  fp16 Karatsuba pair x = x1 + x2 (x1 = fp16(x), x2 = fp16(x - x1)); same
  total DMA bytes as fp32. W likewise (w1, w2), pre-transposed into
  (128, 64)-chunk layout so the kernel needs no on-chip transposes at all.
- The gating matmul runs 3 fp16 passes per K-chunk (x1w1 + x1w2 + x2w1,
  dropping the ~2^-22 x2w2 term), accumulating exact fp32 in PSUM. fp16
  streams at 1 cyc/row: W.T chunks are the stationary operand (64-column
  fast weight loads) and x.T streams 1024 tokens per matmul, so the whole
  gating GEMM is 24 matmuls per 1024-token group. The (64, 1024) logits.T
  is copied to SBUF and transposed back to token-partitioned 128x64 tiles
  on the PE (one small transpose per tile).
- Per token tile: masked logits copy + exp (with fused accumulate for the
  softmax denominator) on ACT, top-8 max/max_index on DVE; probs and the
  logits/probs output DMAs stream per group; small top-2 weight/index tail.

Within a core, token tau = n*128 + p (n tile 0..31, p partition); the mask is
pre-laid-out and outputs are unscrambled on the host accordingly.
"""

import sys

sys.path.insert(0, "/opt/trn_rl_repo")

from contextlib import ExitStack

import numpy as np

import concourse.bass as bass
import concourse.mybir as mybir
import concourse.tile as tile
from concourse import bacc
from concourse.bass_utils import run_bass_kernel_spmd
from concourse.masks import make_identity

N_CORES = 8
B, T, D, E = 4, 8192, 1024, 64
P = 128                   # SBUF partitions
NTOK = B * T // N_CORES   # 4096 tokens per core
S = NTOK // P             # 32 token-tiles
SG = 4                    # token-tiles per pipeline group
G = S // SG               # 8 groups
DC = D // 128             # 8 contraction chunks
TOKG = P * SG             # 512 tokens per group (one PSUM bank of logits.T)

f16 = mybir.dt.float16
f32 = mybir.dt.float32
i32 = mybir.dt.int32
u32 = mybir.dt.uint32
ACT_COPY = mybir.ActivationFunctionType.Copy
ACT_EXP = mybir.ActivationFunctionType.Exp


def build_nc():
    nc = bacc.Bacc("TRN2", target_bir_lowering=False, debug=False)

    # x split halves, d-major: rows 0..1023 = x1 chunks, 1024..2047 = x2
    xt_d = nc.dram_tensor("xt", [2 * D, NTOK], f16, kind="ExternalInput").ap()
    # W split halves, pre-transposed to chunk layout [2][8][128][64]
    w_d = nc.dram_tensor("w", [2, DC, P, E], f16, kind="ExternalInput").ap()
    m_d = nc.dram_tensor("mask", [P, S], i32, kind="ExternalInput").ap()
    ew_d = nc.dram_tensor("ew", [P, S, 2], f32, kind="ExternalOutput").ap()
    ei_d = nc.dram_tensor("ei", [P, S, 2], i32, kind="ExternalOutput").ap()
    lg_d = nc.dram_tensor("logits", [P, S, E], f32, kind="ExternalOutput").ap()
    pr_d = nc.dram_tensor("probs", [P, S, E], f32, kind="ExternalOutput").ap()

    xt_v = xt_d.rearrange("(a p) t -> p a t", p=P)   # (128, 16, 4096)
    w_v = w_d.rearrange("h c p e -> p h c e")        # (128, 2, 8, 64)

    with tile.TileContext(nc) as tc, ExitStack() as ctx:
        const = ctx.enter_context(tc.tile_pool(name="const", bufs=1))
        persist = ctx.enter_context(tc.tile_pool(name="persist", bufs=1))
        xpool = ctx.enter_context(tc.tile_pool(name="xp", bufs=2))
        ltpool = ctx.enter_context(tc.tile_pool(name="ltp", bufs=2))
        tailp = ctx.enter_context(tc.tile_pool(name="tailp", bufs=1))
        ps_lt = ctx.enter_context(tc.tile_pool(name="ps_lt", bufs=2, space="PSUM"))
        ps_lg = ctx.enter_context(tc.tile_pool(name="ps_lg", bufs=4, space="PSUM"))

        wt = const.tile([P, 2, DC, E], f16)
        nc.sync.dma_start(wt[:], w_v)

        # identity for the small back-transposes; built on gpsimd, observed by
        # the PE once via a warm-up transpose so real matmults keep their
        # single fused-LW wait slot free.
        ident = const.tile([E, E], f32)
        make_identity(nc, ident)
        pwm = ps_lg.tile([P, E], f32, tag="pslg")
        nc.tensor.transpose(pwm[0:E, :], ident[:], ident[:])

        # ---- mask (host-laid-out as (p, n)) -> f32; mm1 = m-1 ----
        m_i = const.tile([P, S], i32)
        nc.sync.dma_start(m_i[:], m_d)
        m_sb = const.tile([P, S], f32)
        nc.vector.tensor_copy(m_sb[:], m_i[:])
        mm1 = const.tile([P, S], f32)
        nc.vector.tensor_scalar_add(mm1[:], m_sb[:], -1.0)

        e_sb = persist.tile([P, S, E], f32)    # exp(masked logits)
        sums = persist.tile([P, S], f32)       # softmax denominators
        lg_sb = persist.tile([P, S, E], f32)   # masked logits (output)
        pr_sb = persist.tile([P, S, E], f32)   # masked probs (output)
        mx_all = persist.tile([P, S, 8], f32)  # top-8 values per token
        ix_all = persist.tile([P, S, 8], u32)  # top-8 indices per token
        rm = persist.tile([P, S], f32)         # mask/sum softmax scale

        for g in range(G):
            t0 = g * TOKG
            s0 = g * SG
            gsl = slice(s0, s0 + SG)
            # Split the group load so the x1 passes can start after half the
            # bytes have landed; separate tiles keep the dependencies clean.
            x1g = xpool.tile([P, DC, TOKG], f16, tag="x1g")
            nc.sync.dma_start(x1g[:], xt_v[:, 0:DC, t0:t0 + TOKG])
            x2g = xpool.tile([P, DC, TOKG], f16, tag="x2g")
            nc.sync.dma_start(x2g[:], xt_v[:, DC:2 * DC, t0:t0 + TOKG])

            # gating GEMM, transposed-out: logits.T (64, 512) accumulates 24
            # fp16 chunk matmuls (3 Karatsuba passes x 8 K-chunks), each
            # streaming 512 tokens. x1 passes first.
            plt = ps_lt.tile([E, TOKG], f32, tag="pslt")
            n_mm = 3 * DC
            i_mm = 0
            for c in range(DC):
                for h in (0, 1):
                    nc.tensor.matmul(
                        plt[:], wt[:, h, c, :], x1g[:, c, :],
                        start=(i_mm == 0), stop=False)
                    i_mm += 1
            for c in range(DC):
                nc.tensor.matmul(
                    plt[:], wt[:, 0, c, :], x2g[:, c, :],
                    start=False, stop=(i_mm == n_mm - 1))
                i_mm += 1
            lt_sb = ltpool.tile([E, TOKG], f32, tag="lt")
            nc.scalar.activation(lt_sb[:], plt[:], ACT_COPY)

            # back-transpose all SG tiles into ONE PSUM bank so the masking,
            # exp, and softmax-sum run as single batched ops per group.
            plg = ps_lg.tile([P, SG, E], f32, tag="pslg")  # (128, 4, 64)
            for sl in range(SG):
                nc.tensor.transpose(plg[:, sl, :],
                                    lt_sb[:, sl * P:(sl + 1) * P], ident[:])
            mb = m_sb[:, gsl].unsqueeze(2).broadcast_to([P, SG, E])
            nc.vector.tensor_mul(lg_sb[:, gsl, :], plg[:], mb)
            nc.scalar.activation(e_sb[:, gsl, :], lg_sb[:, gsl, :], ACT_EXP)
            nc.vector.reduce_sum(sums[:, gsl], e_sb[:, gsl, :],
                                 axis=mybir.AxisListType.X)
            for sl in range(SG):
                s_abs = s0 + sl
                nc.vector.max(mx_all[:, s_abs, :], e_sb[:, s_abs, :])
                nc.vector.max_index(ix_all[:, s_abs, :], mx_all[:, s_abs, :],
                                    e_sb[:, s_abs, :])

            # per-group probs + streaming output of logits/probs
            nc.vector.reciprocal(rm[:, gsl], sums[:, gsl])
            nc.vector.tensor_mul(rm[:, gsl], rm[:, gsl], m_sb[:, gsl])
            nc.vector.tensor_mul(
                pr_sb[:, gsl, :], e_sb[:, gsl, :],
                rm[:, gsl].unsqueeze(2).broadcast_to([P, SG, E]))
            nc.sync.dma_start(lg_d[:, gsl, :], lg_sb[:, gsl, :])
            nc.sync.dma_start(pr_d[:, gsl, :], pr_sb[:, gsl, :])

        # ---- batched top-2 weight/index tail ----
        s12 = tailp.tile([P, S], f32)
        nc.vector.tensor_add(s12[:], mx_all[:, :, 0], mx_all[:, :, 1])
        r12 = tailp.tile([P, S], f32)
        nc.vector.reciprocal(r12[:], s12[:])
        rw = tailp.tile([P, S], f32)
        nc.vector.tensor_mul(rw[:], r12[:], m_sb[:])
        ew_sb = tailp.tile([P, S, 2], f32)
        nc.vector.tensor_mul(ew_sb[:, :, 0], mx_all[:, :, 0], rw[:])
        nc.vector.tensor_mul(ew_sb[:, :, 1], mx_all[:, :, 1], rw[:])

        if_sb = tailp.tile([P, S, 2], f32)
        nc.vector.tensor_copy(if_sb[:], ix_all[:, :, 0:2])
        nc.vector.tensor_mul(if_sb[:], if_sb[:],
                             m_sb[:].unsqueeze(2).broadcast_to([P, S, 2]))
        nc.vector.tensor_add(if_sb[:], if_sb[:],
                             mm1[:].unsqueeze(2).broadcast_to([P, S, 2]))
        ei_sb = tailp.tile([P, S, 2], i32)
        nc.vector.tensor_copy(ei_sb[:], if_sb[:])

        nc.sync.dma_start(ew_d, ew_sb[:])
        nc.sync.dma_start(ei_d, ei_sb[:])

    nc.compile()
    return nc


_NC_CACHE = {}


def get_nc():
    if "nc" not in _NC_CACHE:
        _NC_CACHE["nc"] = build_nc()
    return _NC_CACHE["nc"]


def make_in_maps(x, W, x_mask):
    x = np.asarray(x, dtype=np.float32).reshape(B * T, D)
    W = np.asarray(W, dtype=np.float32)
    m = np.asarray(x_mask, dtype=np.int32).reshape(B * T)

    w1 = W.astype(np.float16)
    w2 = (W - w1.astype(np.float32)).astype(np.float16)
    # chunk layout [2][c][p][e]: w_d[h, c, p, e] = wh[e, c*128 + p]
    wck = np.stack([
        w1.T.reshape(DC, P, E),
        w2.T.reshape(DC, P, E),
    ]).astype(np.float16)
    wck = np.ascontiguousarray(wck)

    in_maps = []
    for c in range(N_CORES):
        xs = x[c * NTOK:(c + 1) * NTOK]                    # (4096, 1024)
        ms = m[c * NTOK:(c + 1) * NTOK]                    # (4096,)
        x1 = xs.astype(np.float16)
        x2 = (xs - x1.astype(np.float32)).astype(np.float16)
        xt = np.empty((2 * D, NTOK), dtype=np.float16)
        xt[:D] = x1.T
        xt[D:] = x2.T
        in_maps.append({
            "xt": xt,
            "w": wck,
            # token tau = n*128 + p  ->  mask tile [p, n]
            "mask": np.ascontiguousarray(ms.reshape(S, P).T),
        })
    return in_maps


def _unscramble(a):
    # kernel writes (p, n, k); token tau = n*128 + p
    return a.transpose(1, 0, 2).reshape(NTOK, a.shape[2])


def assemble(results):
    ew = np.concatenate([_unscramble(r["ew"]) for r in results], axis=0)
    ei = np.concatenate([_unscramble(r["ei"]) for r in results], axis=0)
    lg = np.concatenate([_unscramble(r["logits"]) for r in results], axis=0)
    pr = np.concatenate([_unscramble(r["probs"]) for r in results], axis=0)
    return (
        np.ascontiguousarray(ew.reshape(B, T, 2), dtype=np.float32),
        np.ascontiguousarray(ei.reshape(B, T, 2), dtype=np.int32),
        np.ascontiguousarray(lg.reshape(B, T, E), dtype=np.float32),
        np.ascontiguousarray(pr.reshape(B, T, E), dtype=np.float32),
    )


def kernel(x, W, x_mask):
    nc = get_nc()
    in_maps = make_in_maps(x, W, x_mask)
    res = run_bass_kernel_spmd(nc, in_maps, list(range(N_CORES))).results
    return assemble(res)


# revision 34
# speedup vs baseline: 1.0598x; 1.0598x over previous
"""MoE router kernel for Trainium2 (Bass/Tile), SPMD over 8 NeuronCores.

Computes, for x:(B,T,D) f32, W:(E,D) f32, x_mask:(B,T) i32 {0,1}:
  m       = x_mask[..., None]
  logits  = (x*m) @ W.T * m            # (B,T,E)
  probs   = softmax(logits, -1)
  ew, ei  = top2(probs);  ew /= ew.sum(-1, keepdims=True);  ew *= m
  ei      = where(m, ei, -1)
  probs   = probs * m
returns (ew, ei, logits, probs).

Sharding: data-parallel over B*T tokens, 4096 tokens per core, W replicated.

Strategy:
- Host pre-transposes each core's x shard to d-major and splits it into an
  fp16 Karatsuba pair x = x1 + x2 (x1 = fp16(x), x2 = fp16(x - x1)); same
  total DMA bytes as fp32. W likewise (w1, w2), pre-transposed into
  (128, 64)-chunk layout so the kernel needs no on-chip transposes at all.
- The gating matmul runs 3 fp16 passes per K-chunk (x1w1 + x1w2 + x2w1,
  dropping the ~2^-22 x2w2 term), accumulating exact fp32 in PSUM. fp16
  streams at 1 cyc/row: W.T chunks are the stationary operand (64-column
  fast weight loads) and x.T streams 1024 tokens per matmul, so the whole
  gating GEMM is 24 matmuls per 1024-token group. The (64, 1024) logits.T
  is copied to SBUF and transposed back to token-partitioned 128x64 tiles
  on the PE (one small transpose per tile).
- Per token tile: masked logits copy + exp (with fused accumulate for the
  softmax denominator) on ACT, top-8 max/max_index on DVE; probs and the
  logits/probs output DMAs stream per group; small top-2 weight/index tail.

Within a core, token tau = n*128 + p (n tile 0..31, p partition); the mask is
pre-laid-out and outputs are unscrambled on the host accordingly.
"""

import sys

sys.path.insert(0, "/opt/trn_rl_repo")

from contextlib import ExitStack

import numpy as np

import concourse.bass as bass
import concourse.mybir as mybir
import concourse.tile as tile
from concourse import bacc
from concourse.bass_utils import run_bass_kernel_spmd
from concourse.masks import make_identity

N_CORES = 8
B, T, D, E = 4, 8192, 1024, 64
P = 128                   # SBUF partitions
NTOK = B * T // N_CORES   # 4096 tokens per core
S = NTOK // P             # 32 token-tiles
SG = 4                    # token-tiles per pipeline group
G = S // SG               # 8 groups
DC = D // 128             # 8 contraction chunks
TOKG = P * SG             # 512 tokens per group (one PSUM bank of logits.T)

f16 = mybir.dt.float16
f32 = mybir.dt.float32
i32 = mybir.dt.int32
u32 = mybir.dt.uint32
ACT_COPY = mybir.ActivationFunctionType.Copy
ACT_EXP = mybir.ActivationFunctionType.Exp


def build_nc():
    nc = bacc.Bacc("TRN2", target_bir_lowering=False, debug=False)

    # x split halves, d-major: rows 0..1023 = x1 chunks, 1024..2047 = x2
    xt_d = nc.dram_tensor("xt", [2 * D, NTOK], f16, kind="ExternalInput").ap()
    # W split halves, pre-transposed to chunk layout [2][8][128][64]
    w_d = nc.dram_tensor("w", [2, DC, P, E], f16, kind="ExternalInput").ap()
    m_d = nc.dram_tensor("mask", [P, S], i32, kind="ExternalInput").ap()
    ew_d = nc.dram_tensor("ew", [P, S, 2], f32, kind="ExternalOutput").ap()
    ei_d = nc.dram_tensor("ei", [P, S, 2], i32, kind="ExternalOutput").ap()
    lg_d = nc.dram_tensor("logits", [P, S, E], f32, kind="ExternalOutput").ap()
    pr_d = nc.dram_tensor("probs", [P, S, E], f32, kind="ExternalOutput").ap()

    xt_v = xt_d.rearrange("(a p) t -> p a t", p=P)   # (128, 16, 4096)
    w_v = w_d.rearrange("h c p e -> p h c e")        # (128, 2, 8, 64)

    with tile.TileContext(nc) as tc, ExitStack() as ctx:
        const = ctx.enter_context(tc.tile_pool(name="const", bufs=1))
        persist = ctx.enter_context(tc.tile_pool(name="persist", bufs=1))
        xpool = ctx.enter_context(tc.tile_pool(name="xp", bufs=3))
        ltpool = ctx.enter_context(tc.tile_pool(name="ltp", bufs=2))
        tailp = ctx.enter_context(tc.tile_pool(name="tailp", bufs=1))
        ps_lt = ctx.enter_context(tc.tile_pool(name="ps_lt", bufs=2, space="PSUM"))
        ps_lg = ctx.enter_context(tc.tile_pool(name="ps_lg", bufs=4, space="PSUM"))

        # first group's x load goes out before anything else on the SP queue
        xg0 = xpool.tile([P, 2 * DC, TOKG], f16, tag="xtg")
        nc.sync.dma_start(xg0[:], xt_v[:, :, 0:TOKG])

        wt = const.tile([P, 2, DC, E], f16)
        nc.sync.dma_start(wt[:], w_v)

        # identity for the small back-transposes; built on gpsimd, observed by
        # the PE once via a warm-up transpose so real matmults keep their
        # single fused-LW wait slot free.
        ident = const.tile([E, E], f32)
        make_identity(nc, ident)
        pwm = ps_lg.tile([P, E], f32, tag="pslg")
        nc.tensor.transpose(pwm[0:E, :], ident[:], ident[:])

        # ---- mask (host-laid-out as (p, n)) -> f32; mm1 = m-1 ----
        m_i = const.tile([P, S], i32)
        nc.sync.dma_start(m_i[:], m_d)
        m_sb = const.tile([P, S], f32)
        nc.vector.tensor_copy(m_sb[:], m_i[:])
        mm1 = const.tile([P, S], f32)
        nc.vector.tensor_scalar_add(mm1[:], m_sb[:], -1.0)

        e_sb = persist.tile([P, S, E], f32)    # exp(masked logits)
        sums = persist.tile([P, S], f32)       # softmax denominators
        lg_sb = persist.tile([P, S, E], f32)   # masked logits (output)
        pr_sb = persist.tile([P, S, E], f32)   # masked probs (output)
        mx_all = persist.tile([P, S, 8], f32)  # top-8 values per token
        ix_all = persist.tile([P, S, 8], u32)  # top-8 indices per token
        rm = persist.tile([P, S], f32)         # mask/sum softmax scale

        for g in range(G):
            t0 = g * TOKG
            s0 = g * SG
            gsl = slice(s0, s0 + SG)
            if g == 0:
                xtg = xg0
            else:
                xtg = xpool.tile([P, 2 * DC, TOKG], f16, tag="xtg")
                nc.sync.dma_start(xtg[:], xt_v[:, :, t0:t0 + TOKG])

            # gating GEMM, transposed-out: logits.T (64, 512) accumulates 24
            # fp16 chunk matmuls (3 Karatsuba passes x 8 K-chunks), each
            # streaming 512 tokens.
            plt = ps_lt.tile([E, TOKG], f32, tag="pslt")
            n_mm = 3 * DC
            i_mm = 0
            for c in range(DC):
                for h, a in ((0, c), (1, c), (0, DC + c)):
                    nc.tensor.matmul(
                        plt[:], wt[:, h, c, :], xtg[:, a, :],
                        start=(i_mm == 0), stop=(i_mm == n_mm - 1))
                    i_mm += 1
            lt_sb = ltpool.tile([E, TOKG], f32, tag="lt")
            nc.scalar.activation(lt_sb[:], plt[:], ACT_COPY)

            # back-transpose all SG tiles into ONE PSUM bank so the masking,
            # exp, and softmax-sum run as single batched ops per group.
            plg = ps_lg.tile([P, SG, E], f32, tag="pslg")  # (128, 4, 64)
            for sl in range(SG):
                nc.tensor.transpose(plg[:, sl, :],
                                    lt_sb[:, sl * P:(sl + 1) * P], ident[:])
            mb = m_sb[:, gsl].unsqueeze(2).broadcast_to([P, SG, E])
            nc.vector.tensor_mul(lg_sb[:, gsl, :], plg[:], mb)
            nc.scalar.activation(e_sb[:, gsl, :], lg_sb[:, gsl, :], ACT_EXP)
            nc.vector.reduce_sum(sums[:, gsl], e_sb[:, gsl, :],
                                 axis=mybir.AxisListType.X)
            for sl in range(SG):
                s_abs = s0 + sl
                nc.vector.max(mx_all[:, s_abs, :], e_sb[:, s_abs, :])
                nc.vector.max_index(ix_all[:, s_abs, :], mx_all[:, s_abs, :],
                                    e_sb[:, s_abs, :])

            # per-group probs + streaming output of logits/probs
            nc.vector.reciprocal(rm[:, gsl], sums[:, gsl])
            nc.vector.tensor_mul(rm[:, gsl], rm[:, gsl], m_sb[:, gsl])
            nc.vector.tensor_mul(
                pr_sb[:, gsl, :], e_sb[:, gsl, :],
                rm[:, gsl].unsqueeze(2).broadcast_to([P, SG, E]))
            # outputs ride the scalar-engine HWDGE queue so they never delay
            # the SP queue's input loads
            nc.scalar.dma_start(lg_d[:, gsl, :], lg_sb[:, gsl, :])
            nc.scalar.dma_start(pr_d[:, gsl, :], pr_sb[:, gsl, :])

        # ---- batched top-2 weight/index tail ----
        s12 = tailp.tile([P, S], f32)
        nc.vector.tensor_add(s12[:], mx_all[:, :, 0], mx_all[:, :, 1])
        r12 = tailp.tile([P, S], f32)
        nc.vector.reciprocal(r12[:], s12[:])
        rw = tailp.tile([P, S], f32)
        nc.vector.tensor_mul(rw[:], r12[:], m_sb[:])
        ew_sb = tailp.tile([P, S, 2], f32)
        nc.vector.tensor_mul(ew_sb[:, :, 0], mx_all[:, :, 0], rw[:])
        nc.vector.tensor_mul(ew_sb[:, :, 1], mx_all[:, :, 1], rw[:])

        if_sb = tailp.tile([P, S, 2], f32)
        nc.vector.tensor_copy(if_sb[:], ix_all[:, :, 0:2])
        nc.vector.tensor_mul(if_sb[:], if_sb[:],
                             m_sb[:].unsqueeze(2).broadcast_to([P, S, 2]))
        nc.vector.tensor_add(if_sb[:], if_sb[:],
                             mm1[:].unsqueeze(2).broadcast_to([P, S, 2]))
        ei_sb = tailp.tile([P, S, 2], i32)
        nc.vector.tensor_copy(ei_sb[:], if_sb[:])

        nc.scalar.dma_start(ew_d, ew_sb[:])
        nc.scalar.dma_start(ei_d, ei_sb[:])

    nc.compile()
    return nc


_NC_CACHE = {}


def get_nc():
    if "nc" not in _NC_CACHE:
        _NC_CACHE["nc"] = build_nc()
    return _NC_CACHE["nc"]


def make_in_maps(x, W, x_mask):
    x = np.asarray(x, dtype=np.float32).reshape(B * T, D)
    W = np.asarray(W, dtype=np.float32)
    m = np.asarray(x_mask, dtype=np.int32).reshape(B * T)

    w1 = W.astype(np.float16)
    w2 = (W - w1.astype(np.float32)).astype(np.float16)
    # chunk layout [2][c][p][e]: w_d[h, c, p, e] = wh[e, c*128 + p]
    wck = np.stack([
        w1.T.reshape(DC, P, E),
        w2.T.reshape(DC, P, E),
    ]).astype(np.float16)
    wck = np.ascontiguousarray(wck)

    in_maps = []
    for c in range(N_CORES):
        xs = x[c * NTOK:(c + 1) * NTOK]                    # (4096, 1024)
        ms = m[c * NTOK:(c + 1) * NTOK]                    # (4096,)
        x1 = xs.astype(np.float16)
        x2 = (xs - x1.astype(np.float32)).astype(np.float16)
        xt = np.empty((2 * D, NTOK), dtype=np.float16)
        xt[:D] = x1.T
        xt[D:] = x2.T
        in_maps.append({
            "xt": xt,
            "w": wck,
            # token tau = n*128 + p  ->  mask tile [p, n]
            "mask": np.ascontiguousarray(ms.reshape(S, P).T),
        })
    return in_maps


def _unscramble(a):
    # kernel writes (p, n, k); token tau = n*128 + p
    return a.transpose(1, 0, 2).reshape(NTOK, a.shape[2])


def assemble(results):
    ew = np.concatenate([_unscramble(r["ew"]) for r in results], axis=0)
    ei = np.concatenate([_unscramble(r["ei"]) for r in results], axis=0)
    lg = np.concatenate([_unscramble(r["logits"]) for r in results], axis=0)
    pr = np.concatenate([_unscramble(r["probs"]) for r in results], axis=0)
    return (
        np.ascontiguousarray(ew.reshape(B, T, 2), dtype=np.float32),
        np.ascontiguousarray(ei.reshape(B, T, 2), dtype=np.int32),
        np.ascontiguousarray(lg.reshape(B, T, E), dtype=np.float32),
        np.ascontiguousarray(pr.reshape(B, T, E), dtype=np.float32),
    )


def kernel(x, W, x_mask):
    nc = get_nc()
    in_maps = make_in_maps(x, W, x_mask)
    res = run_bass_kernel_spmd(nc, in_maps, list(range(N_CORES))).results
    return assemble(res)
